# revision 10
# baseline (speedup 1.0000x reference)
"""Trainium2 Bass kernel for GCFAgg-style block:
    q1 = x@W1.T+b1; q2 = x@W2.T+b2; r = x@WR.T+br
    out = (q1 @ q2.T) @ r        (per batch, no softmax)

Key algebraic restructuring: with x_aug = [x | 1] and W*_aug = [W* | b*],
    out = x_aug @ (Khat @ (x_aug.T @ x_aug) @ Rhat)
where Khat = W1_aug.T @ W2_aug and Rhat = WR_aug.T are tiny host-precomputed
matrices. The device only computes G = x_aug.T @ x_aug (per batch) plus a
small [640]^2-sized chain and the final projection — ~4.5 GFLOP/core instead
of ~41 GFLOP/core for the naive N x N similarity materialization.

Sharding: batch dim B=8, one batch per NeuronCore (data parallel, 8 cores).

Self-contained: hardcodes shapes from the problem spec
(x: [8, 4096, 512] f32; W*: [512, 512]; b*: [512]).
"""
import os
import sys

sys.path.insert(0, "/opt/trn_rl_repo")

import numpy as np
import ml_dtypes

import concourse.bass as bass
import concourse.mybir as mybir
import concourse.tile as tile
from concourse import bacc
from concourse.bass_utils import run_bass_kernel_spmd

B = 8          # batch -> one per core
N = 4096       # tokens per batch
D = 512        # model dim
GP = 640       # augmented dim 513 padded to 5*128
NCHUNK = GP // 128   # 5
NT = N // 128        # 32 row tiles
N_CORES = 8

F32 = mybir.dt.float32
F32R = mybir.dt.float32r
BF16 = mybir.dt.bfloat16

# mode: "f32r" (fp32 storage, single-pass reduced-precision matmul),
#       "bf16" (bf16 storage+matmul), "f32" (full-precision 4-pass matmul)
MODE = os.environ.get("GCF_MODE", "f32r")

_built = {}


def _build(mode):
    if mode in _built:
        return _built[mode]

    # Storage dtype IS the matmul dtype: the BIR verifier requires fp32r
    # matmul inputs to be produced (DMA'd/copied) as fp32r, so tiles are
    # declared in the matmul dtype directly.
    if mode == "bf16":
        big_mm = BF16
    elif mode == "f32":
        big_mm = F32
    else:
        big_mm = F32R
    big_store = big_mm
    chain_mm = F32 if mode == "f32" else F32R

    def mm_ap(ap, dt):
        return ap if ap.dtype == dt else ap.bitcast(dt)

    nc = bacc.Bacc("TRN2", target_bir_lowering=False, debug=False,
                   num_devices=N_CORES)

    xa_d = nc.dram_tensor("xa", (N, GP), big_store, kind="ExternalInput")
    xat_d = nc.dram_tensor("xat", (NT, 128, NCHUNK, 128), big_store,
                           kind="ExternalInput")
    khatT_d = nc.dram_tensor("khatT", (GP, GP), chain_mm, kind="ExternalInput")
    rhat_d = nc.dram_tensor("rhat", (GP, D), chain_mm, kind="ExternalInput")
    out_d = nc.dram_tensor("out", (N, D), F32, kind="ExternalOutput")

    with tile.TileContext(nc) as tc:
        with (
            tc.tile_pool(name="xa", bufs=4) as xa_pool,
            tc.tile_pool(name="xat", bufs=16) as xat_pool,
            tc.tile_pool(name="const", bufs=1) as const_pool,
            tc.tile_pool(name="gsb", bufs=1) as g_pool,
            tc.tile_pool(name="chain", bufs=1) as chain_pool,
            tc.tile_pool(name="outsb", bufs=4) as out_pool,
        ):
            # ---- constants ----
            khat_sb = [const_pool.tile([128, GP], chain_mm, tag=f"khat{c}", name=f"khat{c}")
                       for c in range(NCHUNK)]
            rhat_sb = [const_pool.tile([128, D], chain_mm, tag=f"rhat{c}", name=f"rhat{c}")
                       for c in range(NCHUNK)]
            for c in range(NCHUNK):
                nc.sync.dma_start(khat_sb[c][:], khatT_d.ap()[c * 128:(c + 1) * 128, :])
                nc.sync.dma_start(rhat_sb[c][:], rhat_d.ap()[c * 128:(c + 1) * 128, :])
            # memset can't emit f32r; produce constants as f32 then DVE-copy
            # (the copy rounds/retypes to f32r)
            const_f32 = const_pool.tile([128, 128], F32, tag="cf32")
            nc.vector.memset(const_f32[:], 0.0)
            one_f32 = const_pool.tile([1, 2], F32, tag="onef32")
            nc.vector.memset(one_f32[0:1, 0:1], 1.0)
            nc.vector.memset(one_f32[0:1, 1:2], float(N))
            ones_sb = const_pool.tile([1, 1], chain_mm, tag="ones")
            nc.vector.tensor_copy(ones_sb[:], one_f32[0:1, 0:1])

            # ---- phase 1: G = xa^T @ xa, accumulated over 32 row tiles ----
            # G columns 0:512 via 5 persistent PSUM banks; the augmented
            # columns (512:640) are reconstructed from G's symmetry below.
            g_sb = [g_pool.tile([128, GP], chain_mm, tag=f"g{c}", name=f"g{c}")
                    for c in range(NCHUNK)]
            with tc.tile_pool(name="psG", bufs=1, space="PSUM") as psG_pool:
                ps_ga = [psG_pool.tile([128, D], F32, tag=f"ga{c}", name=f"ga{c}")
                         for c in range(NCHUNK)]
                for t in range(NT):
                    xa_t = xa_pool.tile([128, GP], big_store, tag="xa")
                    nc.sync.dma_start(xa_t[:], xa_d.ap()[t * 128:(t + 1) * 128, :])
                    for c in range(NCHUNK):
                        nc.tensor.matmul(
                            ps_ga[c][:],
                            mm_ap(xa_t[:, c * 128:(c + 1) * 128], big_mm),
                            mm_ap(xa_t[:, 0:D], big_mm),
                            start=(t == 0), stop=(t == NT - 1),
                        )

                for c in range(NCHUNK):
                    nc.vector.tensor_copy(g_sb[c][:, 0:D], ps_ga[c][:])
                    nc.vector.tensor_copy(g_sb[c][:, D:GP], const_f32[:])
                # column 512 of G_aug = row 512 (sx) transposed; tiny matmuls
                # against [1,1] ones give the [1,128] -> [128,1] transposes.
                # (plain f32 matmuls: fp32r has dst-pattern ISA restrictions
                # that reject the narrow [128,1] outputs; K=1,N=1 is trivial)
                ps_sx = psG_pool.tile([128, 4], F32, tag="sx")
                for c in range(4):
                    nc.tensor.matmul(
                        ps_sx[:, c:c + 1],
                        mm_ap(g_sb[4][0:1, c * 128:(c + 1) * 128], F32),
                        mm_ap(ones_sb[0:1, 0:1], F32),
                    )
                for c in range(4):
                    nc.vector.tensor_copy(g_sb[c][:, D:D + 1], ps_sx[:, c:c + 1])
                # G_aug[512, 512] = sum of ones column = N
                nc.vector.tensor_copy(g_sb[4][0:1, D:D + 1], one_f32[0:1, 1:2])

            # ---- phase 2: P = Khat @ G @ Rhat  (small chain) ----
            with tc.tile_pool(name="psC", bufs=2, space="PSUM") as psC_pool:
                m1_sb = [chain_pool.tile([128, D], chain_mm, tag=f"m1{c}", name=f"m1{c}")
                         for c in range(NCHUNK)]
                for g1 in range(NCHUNK):
                    ps = psC_pool.tile([128, D], F32, tag="chain")
                    for g2 in range(NCHUNK):
                        nc.tensor.matmul(
                            ps[:],
                            mm_ap(g_sb[g2][:, g1 * 128:(g1 + 1) * 128], chain_mm),
                            mm_ap(rhat_sb[g2][:], chain_mm),
                            start=(g2 == 0), stop=(g2 == NCHUNK - 1),
                        )
                    nc.vector.tensor_copy(m1_sb[g1][:], ps[:])

                p_sb = [chain_pool.tile([128, D], big_store, tag=f"p{c}", name=f"p{c}")
                        for c in range(NCHUNK)]
                for g1 in range(NCHUNK):
                    ps = psC_pool.tile([128, D], F32, tag="chain")
                    for g2 in range(NCHUNK):
                        nc.tensor.matmul(
                            ps[:],
                            mm_ap(khat_sb[g2][:, g1 * 128:(g1 + 1) * 128], chain_mm),
                            mm_ap(m1_sb[g2][:], chain_mm),
                            start=(g2 == 0), stop=(g2 == NCHUNK - 1),
                        )
                    nc.vector.tensor_copy(p_sb[g1][:], ps[:])

            # ---- phase 3: out = xa @ P ----
            with tc.tile_pool(name="psO", bufs=4, space="PSUM") as psO_pool:
                for t in range(NT):
                    xat_t = xat_pool.tile([128, NCHUNK, 128], big_store, tag="xat")
                    nc.sync.dma_start(xat_t[:], xat_d.ap()[t])
                    ps = psO_pool.tile([128, D], F32, tag="out")
                    for c in range(NCHUNK):
                        nc.tensor.matmul(
                            ps[:],
                            mm_ap(xat_t[:, c, :], big_mm),
                            mm_ap(p_sb[c][:], big_mm),
                            start=(c == 0), stop=(c == NCHUNK - 1),
                        )
                    ot = out_pool.tile([128, D], F32, tag="ot")
                    nc.vector.tensor_copy(ot[:], ps[:])
                    nc.sync.dma_start(out_d.ap()[t * 128:(t + 1) * 128, :], ot[:])

    nc.compile()
    _built[mode] = nc
    return nc


def _prep_host(x, Wq1_w, Wq1_b, Wq2_w, Wq2_b, WR_w, WR_b, mode):
    f = np.float32
    W1a = np.concatenate([Wq1_w, Wq1_b[:, None]], axis=1)   # [512, 513]
    W2a = np.concatenate([Wq2_w, Wq2_b[:, None]], axis=1)
    WRa = np.concatenate([WR_w, WR_b[:, None]], axis=1)

    khatT = np.zeros((GP, GP), f)
    khatT[:D + 1, :D + 1] = (
        W2a.T.astype(np.float64) @ W1a.astype(np.float64)
    ).astype(f)
    rhat = np.zeros((GP, D), f)
    rhat[:D + 1, :] = WRa.T

    xa = np.zeros((B, N, GP), f)
    xa[:, :, :D] = x
    xa[:, :, D] = 1.0

    # xat[b, t, p, c, j] = xa[b, t*128+j, c*128+p] — per-(t) contiguous
    # [128, 5, 128] lhsT blocks of x_aug^T.
    xat = np.ascontiguousarray(
        xa.transpose(0, 2, 1)                    # [B, 640, 4096]
          .reshape(B, NCHUNK, 128, NT, 128)      # [B, c, p, t, j]
          .transpose(0, 3, 2, 1, 4)              # [B, t, p, c, j]
    )

    if mode == "bf16":
        bf = ml_dtypes.bfloat16
        xa = xa.astype(bf)
        xat = xat.astype(bf)
    else:
        xa = np.ascontiguousarray(xa)
    return xa, xat, khatT, rhat


def kernel(x, Wq1_w, Wq1_b, Wq2_w, Wq2_b, WR_w, WR_b):
    x = np.asarray(x, dtype=np.float32)
    args = [np.asarray(a, dtype=np.float32)
            for a in (Wq1_w, Wq1_b, Wq2_w, Wq2_b, WR_w, WR_b)]
    xa, xat, khatT, rhat = _prep_host(x, *args, MODE)

    nc = _build(MODE)
    in_maps = [
        {"xa": xa[b], "xat": xat[b], "khatT": khatT, "rhat": rhat}
        for b in range(B)
    ]
    res = run_bass_kernel_spmd(nc, in_maps, core_ids=list(range(N_CORES)))
    return np.stack([res.results[b]["out"] for b in range(B)])


# revision 13
# speedup vs baseline: 1.2389x; 1.2389x over previous
"""Trainium2 Bass kernel for GCFAgg-style block:
    q1 = x@W1.T+b1; q2 = x@W2.T+b2; r = x@WR.T+br
    out = (q1 @ q2.T) @ r        (per batch, no softmax)

Key algebraic restructuring: with x_aug = [x | 1] and W*_aug = [W* | b*],
    out = x_aug @ (Khat @ (x_aug.T @ x_aug) @ Rhat)
where Khat = W1_aug.T @ W2_aug and Rhat = WR_aug.T are tiny host-precomputed
matrices. The device only computes G = x_aug.T @ x_aug (per batch) plus a
small [640]^2-sized chain and the final projection — ~4.5 GFLOP/core instead
of ~41 GFLOP/core for the naive N x N similarity materialization.

Sharding: batch dim B=8, one batch per NeuronCore (data parallel, 8 cores).

Self-contained: hardcodes shapes from the problem spec
(x: [8, 4096, 512] f32; W*: [512, 512]; b*: [512]).
"""
import os
import sys

sys.path.insert(0, "/opt/trn_rl_repo")

import numpy as np
import ml_dtypes

import concourse.bass as bass
import concourse.mybir as mybir
import concourse.tile as tile
from concourse import bacc
from concourse.bass_utils import run_bass_kernel_spmd

B = 8          # batch -> one per core
N = 4096       # tokens per batch
D = 512        # model dim
GP = 640       # augmented dim 513 padded to 5*128
NCHUNK = GP // 128   # 5
NT = N // 128        # 32 row tiles
N_CORES = 8

F32 = mybir.dt.float32
F32R = mybir.dt.float32r
BF16 = mybir.dt.bfloat16

# mode: "f32r" (fp32 storage, single-pass reduced-precision matmul),
#       "bf16" (bf16 storage+matmul), "f32" (full-precision 4-pass matmul)
MODE = os.environ.get("GCF_MODE", "f32r")

_built = {}


def _build(mode):
    if mode in _built:
        return _built[mode]

    # Storage dtype IS the matmul dtype: the BIR verifier requires fp32r
    # matmul inputs to be produced (DMA'd/copied) as fp32r, so tiles are
    # declared in the matmul dtype directly.
    if mode == "bf16":
        big_mm = BF16
    elif mode == "f32":
        big_mm = F32
    else:
        big_mm = F32R
    big_store = big_mm
    chain_mm = F32 if mode == "f32" else F32R

    def mm_ap(ap, dt):
        return ap if ap.dtype == dt else ap.bitcast(dt)

    nc = bacc.Bacc("TRN2", target_bir_lowering=False, debug=False,
                   num_devices=N_CORES)

    xa_d = nc.dram_tensor("xa", (N, GP), big_store, kind="ExternalInput")
    xat_d = nc.dram_tensor("xat", (NT, 128, NCHUNK, 128), big_store,
                           kind="ExternalInput")
    khatT_d = nc.dram_tensor("khatT", (GP, GP), chain_mm, kind="ExternalInput")
    rhat_d = nc.dram_tensor("rhat", (GP, D), chain_mm, kind="ExternalInput")
    # host-precomputed augmented pieces of G_aug (they only involve column
    # sums of x, cheap on host): rows 512:640, and the [*, 512:640] blocks
    gext_d = nc.dram_tensor("gext", (128, GP), chain_mm, kind="ExternalInput")
    augblk_d = nc.dram_tensor("augblk", (4, 128, GP - D), chain_mm,
                              kind="ExternalInput")
    out_d = nc.dram_tensor("out", (N, D), F32, kind="ExternalOutput")

    with tile.TileContext(nc) as tc:
        with (
            tc.tile_pool(name="xa", bufs=4) as xa_pool,
            tc.tile_pool(name="xat", bufs=16) as xat_pool,
            tc.tile_pool(name="const", bufs=1) as const_pool,
            tc.tile_pool(name="gsb", bufs=1) as g_pool,
            tc.tile_pool(name="chain", bufs=1) as chain_pool,
            tc.tile_pool(name="outsb", bufs=4) as out_pool,
        ):
            # ---- constants ----
            khat_sb = [const_pool.tile([128, GP], chain_mm, tag=f"khat{c}", name=f"khat{c}")
                       for c in range(NCHUNK)]
            rhat_sb = [const_pool.tile([128, D], chain_mm, tag=f"rhat{c}", name=f"rhat{c}")
                       for c in range(NCHUNK)]
            # constants go through the (otherwise idle) GpSimd DMA queue so
            # they don't delay the sync-queue xa/xat streams
            for c in range(NCHUNK):
                nc.gpsimd.dma_start(khat_sb[c][:], khatT_d.ap()[c * 128:(c + 1) * 128, :])
                nc.gpsimd.dma_start(rhat_sb[c][:], rhat_d.ap()[c * 128:(c + 1) * 128, :])

            # ---- phase 1: G = xa^T @ xa, accumulated over 32 row tiles ----
            # G columns 0:512 via 5 persistent PSUM banks; the augmented
            # columns (512:640) are reconstructed from G's symmetry below.
            g_sb = [g_pool.tile([128, GP], chain_mm, tag=f"g{c}", name=f"g{c}")
                    for c in range(NCHUNK)]
            # G_aug rows 512:640 and the [:, 512:640] blocks come from host
            nc.gpsimd.dma_start(g_sb[4][:], gext_d.ap()[:])
            for c in range(4):
                nc.gpsimd.dma_start(g_sb[c][:, D:GP], augblk_d.ap()[c])
            with tc.tile_pool(name="psG", bufs=1, space="PSUM") as psG_pool:
                ps_ga = [psG_pool.tile([128, D], F32, tag=f"ga{c}", name=f"ga{c}")
                         for c in range(4)]
                for t in range(NT):
                    xa_t = xa_pool.tile([128, GP], big_store, tag="xa")
                    nc.sync.dma_start(xa_t[:], xa_d.ap()[t * 128:(t + 1) * 128, :])
                    for c in range(4):
                        nc.tensor.matmul(
                            ps_ga[c][:],
                            mm_ap(xa_t[:, c * 128:(c + 1) * 128], big_mm),
                            mm_ap(xa_t[:, 0:D], big_mm),
                            start=(t == 0), stop=(t == NT - 1),
                        )

                for c in range(4):
                    nc.vector.tensor_copy(g_sb[c][:, 0:D], ps_ga[c][:])

            # ---- phase 2: P = Khat @ G @ Rhat  (small chain) ----
            with tc.tile_pool(name="psC", bufs=2, space="PSUM") as psC_pool:
                m1_sb = [chain_pool.tile([128, D], chain_mm, tag=f"m1{c}", name=f"m1{c}")
                         for c in range(NCHUNK)]
                for g1 in range(NCHUNK):
                    ps = psC_pool.tile([128, D], F32, tag="chain")
                    for g2 in range(NCHUNK):
                        nc.tensor.matmul(
                            ps[:],
                            mm_ap(g_sb[g2][:, g1 * 128:(g1 + 1) * 128], chain_mm),
                            mm_ap(rhat_sb[g2][:], chain_mm),
                            start=(g2 == 0), stop=(g2 == NCHUNK - 1),
                        )
                    nc.vector.tensor_copy(m1_sb[g1][:], ps[:])

                p_sb = [chain_pool.tile([128, D], big_store, tag=f"p{c}", name=f"p{c}")
                        for c in range(NCHUNK)]
                for g1 in range(NCHUNK):
                    ps = psC_pool.tile([128, D], F32, tag="chain")
                    for g2 in range(NCHUNK):
                        nc.tensor.matmul(
                            ps[:],
                            mm_ap(khat_sb[g2][:, g1 * 128:(g1 + 1) * 128], chain_mm),
                            mm_ap(m1_sb[g2][:], chain_mm),
                            start=(g2 == 0), stop=(g2 == NCHUNK - 1),
                        )
                    nc.vector.tensor_copy(p_sb[g1][:], ps[:])

            # ---- phase 3: out = xa @ P ----
            with tc.tile_pool(name="psO", bufs=4, space="PSUM") as psO_pool:
                for t in range(NT):
                    xat_t = xat_pool.tile([128, NCHUNK, 128], big_store, tag="xat")
                    nc.sync.dma_start(xat_t[:], xat_d.ap()[t])
                    ps = psO_pool.tile([128, D], F32, tag="out")
                    for c in range(NCHUNK):
                        nc.tensor.matmul(
                            ps[:],
                            mm_ap(xat_t[:, c, :], big_mm),
                            mm_ap(p_sb[c][:], big_mm),
                            start=(c == 0), stop=(c == NCHUNK - 1),
                        )
                    ot = out_pool.tile([128, D], F32, tag="ot")
                    nc.vector.tensor_copy(ot[:], ps[:])
                    nc.gpsimd.dma_start(out_d.ap()[t * 128:(t + 1) * 128, :], ot[:])

    nc.compile()
    _built[mode] = nc
    return nc


def _prep_host(x, Wq1_w, Wq1_b, Wq2_w, Wq2_b, WR_w, WR_b, mode):
    f = np.float32
    W1a = np.concatenate([Wq1_w, Wq1_b[:, None]], axis=1)   # [512, 513]
    W2a = np.concatenate([Wq2_w, Wq2_b[:, None]], axis=1)
    WRa = np.concatenate([WR_w, WR_b[:, None]], axis=1)

    khatT = np.zeros((GP, GP), f)
    khatT[:D + 1, :D + 1] = (
        W2a.T.astype(np.float64) @ W1a.astype(np.float64)
    ).astype(f)
    rhat = np.zeros((GP, D), f)
    rhat[:D + 1, :] = WRa.T

    xa = np.zeros((B, N, GP), f)
    xa[:, :, :D] = x
    xa[:, :, D] = 1.0

    # augmented pieces of G_aug = xa^T @ xa that only need column sums:
    # row block 512:640 (gext) and the [:, 512:640] column blocks (augblk)
    sx = xa.sum(axis=1, dtype=np.float64).astype(f)      # [B, 640]
    gext = np.zeros((B, 128, GP), f)
    gext[:, 0, :] = sx
    augblk = np.zeros((B, 4, 128, GP - D), f)
    augblk[:, :, :, 0] = sx[:, :D].reshape(B, 4, 128)

    # xat[b, t, p, c, j] = xa[b, t*128+j, c*128+p] — per-(t) contiguous
    # [128, 5, 128] lhsT blocks of x_aug^T.
    xat = np.ascontiguousarray(
        xa.transpose(0, 2, 1)                    # [B, 640, 4096]
          .reshape(B, NCHUNK, 128, NT, 128)      # [B, c, p, t, j]
          .transpose(0, 3, 2, 1, 4)              # [B, t, p, c, j]
    )

    if mode == "bf16":
        bf = ml_dtypes.bfloat16
        xa = xa.astype(bf)
        xat = xat.astype(bf)
    else:
        xa = np.ascontiguousarray(xa)
    return xa, xat, khatT, rhat, gext, augblk


def kernel(x, Wq1_w, Wq1_b, Wq2_w, Wq2_b, WR_w, WR_b):
    x = np.asarray(x, dtype=np.float32)
    args = [np.asarray(a, dtype=np.float32)
            for a in (Wq1_w, Wq1_b, Wq2_w, Wq2_b, WR_w, WR_b)]
    xa, xat, khatT, rhat, gext, augblk = _prep_host(x, *args, MODE)

    nc = _build(MODE)
    in_maps = [
        {"xa": xa[b], "xat": xat[b], "khatT": khatT, "rhat": rhat,
         "gext": gext[b], "augblk": augblk[b]}
        for b in range(B)
    ]
    res = run_bass_kernel_spmd(nc, in_maps, core_ids=list(range(N_CORES)))
    return np.stack([res.results[b]["out"] for b in range(B)])


# revision 14
# speedup vs baseline: 1.3824x; 1.1158x over previous
"""Trainium2 Bass kernel for GCFAgg-style block:
    q1 = x@W1.T+b1; q2 = x@W2.T+b2; r = x@WR.T+br
    out = (q1 @ q2.T) @ r        (per batch, no softmax)

Key algebraic restructuring: with x_aug = [x | 1] and W*_aug = [W* | b*],
    out = x_aug @ (Khat @ (x_aug.T @ x_aug) @ Rhat)
where Khat = W1_aug.T @ W2_aug and Rhat = WR_aug.T are tiny host-precomputed
matrices. The device only computes G = x.T @ x (per batch, symmetric — only
upper blocks are computed, lower blocks come from PE transposes) plus a small
[640]^2-sized chain and the final projection out = x @ P + v. That's
~3.5 GFLOP/core instead of ~41 GFLOP/core for the naive N x N similarity
materialization. The augmented row/col of G (column sums of x) and the
constant v-broadcast are folded in from host-precomputed side inputs.

Numerics: fp32r matmuls (fp32 storage, single-pass reduced-precision PE
multiply) giving ~2e-4 relative error vs the fp32 reference — ~10x tighter
than bf16 at only ~10-15% more device time.

Sharding: batch dim B=8, one batch per NeuronCore (data parallel, 8 cores).

Self-contained: hardcodes shapes from the problem spec
(x: [8, 4096, 512] f32; W*: [512, 512]; b*: [512]).
"""
import os
import sys

sys.path.insert(0, "/opt/trn_rl_repo")

import numpy as np
import ml_dtypes

import concourse.bass as bass
import concourse.mybir as mybir
import concourse.tile as tile
from concourse import bacc
from concourse.bass_utils import run_bass_kernel_spmd
from concourse.masks import make_identity

B = 8          # batch -> one per core
N = 4096       # tokens per batch
D = 512        # model dim
GP = 640       # augmented dim 513 padded to 5*128
NCHUNK = GP // 128   # 5
NT = N // 128        # 32 row tiles
N_CORES = 8

F32 = mybir.dt.float32
F32R = mybir.dt.float32r
BF16 = mybir.dt.bfloat16

# mode: "f32r" (fp32 storage, single-pass reduced-precision matmul),
#       "bf16" (bf16 storage+matmul), "f32" (full-precision 4-pass matmul)
MODE = os.environ.get("GCF_MODE", "f32r")

_built = {}


def _build(mode):
    if mode in _built:
        return _built[mode]

    # Storage dtype IS the matmul dtype: the BIR verifier requires fp32r
    # matmul inputs to be produced (DMA'd/copied) as fp32r.
    if mode == "bf16":
        big_mm = BF16
    elif mode == "f32":
        big_mm = F32
    else:
        big_mm = F32R
    big_store = big_mm
    chain_mm = F32 if mode == "f32" else F32R

    def mm_ap(ap, dt):
        return ap if ap.dtype == dt else ap.bitcast(dt)

    nc = bacc.Bacc("TRN2", target_bir_lowering=False, debug=False,
                   num_devices=N_CORES)

    xa_d = nc.dram_tensor("xa", (N, D), big_store, kind="ExternalInput")
    xat_d = nc.dram_tensor("xat", (NT, 128, 4, 128), big_store,
                           kind="ExternalInput")
    khatT_d = nc.dram_tensor("khatT", (GP, GP), chain_mm, kind="ExternalInput")
    rhat_d = nc.dram_tensor("rhat", (GP, D), chain_mm, kind="ExternalInput")
    # host-precomputed augmented pieces of G_aug (they only involve column
    # sums of x, cheap on host): rows 512:640, and the [:, 512:640] blocks
    gext_d = nc.dram_tensor("gext", (128, GP), chain_mm, kind="ExternalInput")
    augblk_d = nc.dram_tensor("augblk", (4, 128, GP - D), chain_mm,
                              kind="ExternalInput")
    out_d = nc.dram_tensor("out", (N, D), F32, kind="ExternalOutput")

    with tile.TileContext(nc) as tc:
        with (
            tc.tile_pool(name="xa", bufs=6) as xa_pool,
            tc.tile_pool(name="xat", bufs=16) as xat_pool,
            tc.tile_pool(name="const", bufs=1) as const_pool,
            tc.tile_pool(name="gsb", bufs=1) as g_pool,
            tc.tile_pool(name="chain", bufs=1) as chain_pool,
            tc.tile_pool(name="outsb", bufs=4) as out_pool,
        ):
            # ---- constants (via the otherwise-idle GpSimd DMA queue so they
            # don't delay the sync-queue xa/xat streams) ----
            khat_sb = [const_pool.tile([128, GP], chain_mm, tag=f"khat{c}",
                                       name=f"khat{c}") for c in range(NCHUNK)]
            rhat_sb = [const_pool.tile([128, D], chain_mm, tag=f"rhat{c}",
                                       name=f"rhat{c}") for c in range(NCHUNK)]
            for c in range(NCHUNK):
                nc.gpsimd.dma_start(khat_sb[c][:], khatT_d.ap()[c * 128:(c + 1) * 128, :])
                nc.gpsimd.dma_start(rhat_sb[c][:], rhat_d.ap()[c * 128:(c + 1) * 128, :])
            ident = const_pool.tile([128, 128], F32, tag="ident")
            make_identity(nc, ident[:])
            ones_row = const_pool.tile([1, 128], F32, tag="ones_row")
            nc.vector.memset(ones_row[:], 1.0)

            # ---- phase 1: G = x^T @ x over 32 row tiles; G is symmetric so
            # only the upper block-triangle is computed on PE ----
            g_sb = [g_pool.tile([128, GP], chain_mm, tag=f"g{c}", name=f"g{c}")
                    for c in range(NCHUNK)]
            # G_aug rows 512:640 and the [:, 512:640] blocks come from host
            nc.gpsimd.dma_start(g_sb[4][:], gext_d.ap()[:])
            for c in range(4):
                nc.gpsimd.dma_start(g_sb[c][:, D:GP], augblk_d.ap()[c])
            with tc.tile_pool(name="psG", bufs=1, space="PSUM") as psG_pool:
                ps_ga = [psG_pool.tile([128, D - c * 128], F32, tag=f"ga{c}",
                                       name=f"ga{c}") for c in range(4)]
                for t in range(NT):
                    xa_t = xa_pool.tile([128, D], big_store, tag="xa")
                    nc.sync.dma_start(xa_t[:], xa_d.ap()[t * 128:(t + 1) * 128, :])
                    for c in range(4):
                        nc.tensor.matmul(
                            ps_ga[c][:],
                            mm_ap(xa_t[:, c * 128:(c + 1) * 128], big_mm),
                            mm_ap(xa_t[:, c * 128:D], big_mm),
                            start=(t == 0), stop=(t == NT - 1),
                        )
                # upper blocks into SBUF
                for c in range(4):
                    nc.vector.tensor_copy(g_sb[c][:, c * 128:D], ps_ga[c][:])
                # lower blocks = transpose of upper (G symmetric)
                for c2 in range(1, 4):
                    for c1 in range(c2):
                        ps_tr = psG_pool.tile([128, 128], F32, tag="tr", bufs=2)
                        nc.tensor.transpose(
                            ps_tr[:],
                            mm_ap(g_sb[c1][:, c2 * 128:(c2 + 1) * 128], F32),
                            ident[:],
                        )
                        nc.vector.tensor_copy(
                            g_sb[c2][:, c1 * 128:(c1 + 1) * 128], ps_tr[:])

            # ---- phase 2: P = Khat @ G @ Rhat  (small chain) ----
            with tc.tile_pool(name="psC", bufs=2, space="PSUM") as psC_pool:
                m1_sb = [chain_pool.tile([128, D], chain_mm, tag=f"m1{c}",
                                         name=f"m1{c}") for c in range(NCHUNK)]
                for g1 in range(NCHUNK):
                    ps = psC_pool.tile([128, D], F32, tag="chain")
                    for g2 in range(NCHUNK):
                        nc.tensor.matmul(
                            ps[:],
                            mm_ap(g_sb[g2][:, g1 * 128:(g1 + 1) * 128], chain_mm),
                            mm_ap(rhat_sb[g2][:], chain_mm),
                            start=(g2 == 0), stop=(g2 == NCHUNK - 1),
                        )
                    nc.vector.tensor_copy(m1_sb[g1][:], ps[:])

                p_sb = [chain_pool.tile([128, D], big_store, tag=f"p{c}",
                                        name=f"p{c}") for c in range(NCHUNK)]
                for g1 in range(NCHUNK):
                    ps = psC_pool.tile([128, D], F32, tag="chain")
                    for g2 in range(NCHUNK):
                        nc.tensor.matmul(
                            ps[:],
                            mm_ap(khat_sb[g2][:, g1 * 128:(g1 + 1) * 128], chain_mm),
                            mm_ap(m1_sb[g2][:], chain_mm),
                            start=(g2 == 0), stop=(g2 == NCHUNK - 1),
                        )
                    nc.vector.tensor_copy(p_sb[g1][:], ps[:])

            # ---- phase 3: out = x @ P[0:512] + v,  v = P_aug[512, :] ----
            with tc.tile_pool(name="psO", bufs=4, space="PSUM") as psO_pool:
                # v broadcast to 128 partitions via a K=1 fp32 matmul
                ps_v = psO_pool.tile([128, D], F32, tag="v")
                nc.tensor.matmul(
                    ps_v[:], ones_row[0:1, :], mm_ap(p_sb[4][0:1, :], F32),
                    start=True, stop=True,
                )
                v_sb = const_pool.tile([128, D], F32, tag="vsb")
                nc.vector.tensor_copy(v_sb[:], ps_v[:])

                for t in range(NT):
                    xat_t = xat_pool.tile([128, 4, 128], big_store, tag="xat")
                    nc.sync.dma_start(xat_t[:], xat_d.ap()[t])
                    ps = psO_pool.tile([128, D], F32, tag="out")
                    for c in range(4):
                        nc.tensor.matmul(
                            ps[:],
                            mm_ap(xat_t[:, c, :], big_mm),
                            mm_ap(p_sb[c][:], big_mm),
                            start=(c == 0), stop=(c == 3),
                        )
                    ot = out_pool.tile([128, D], F32, tag="ot")
                    nc.vector.tensor_add(ot[:], ps[:], v_sb[:])
                    nc.gpsimd.dma_start(out_d.ap()[t * 128:(t + 1) * 128, :], ot[:])

    nc.compile()
    _built[mode] = nc
    return nc


def _prep_host(x, Wq1_w, Wq1_b, Wq2_w, Wq2_b, WR_w, WR_b, mode):
    f = np.float32
    W1a = np.concatenate([Wq1_w, Wq1_b[:, None]], axis=1)   # [512, 513]
    W2a = np.concatenate([Wq2_w, Wq2_b[:, None]], axis=1)
    WRa = np.concatenate([WR_w, WR_b[:, None]], axis=1)

    khatT = np.zeros((GP, GP), f)   # Khat^T = W2a^T @ W1a, padded
    khatT[:D + 1, :D + 1] = (
        W2a.T.astype(np.float64) @ W1a.astype(np.float64)
    ).astype(f)
    rhat = np.zeros((GP, D), f)     # Rhat = WRa^T, padded
    rhat[:D + 1, :] = WRa.T

    # augmented pieces of G_aug = xa^T @ xa that only need column sums of x
    sx = x.sum(axis=1, dtype=np.float64).astype(f)       # [B, 512]
    gext = np.zeros((B, 128, GP), f)                     # G_aug rows 512:640
    gext[:, 0, :D] = sx
    gext[:, 0, D] = float(N)
    augblk = np.zeros((B, 4, 128, GP - D), f)            # G_aug[:512, 512:640]
    augblk[:, :, :, 0] = sx.reshape(B, 4, 128)

    # xat[b, t, p, c, j] = x[b, t*128+j, c*128+p] — per-(t) contiguous
    # [128, 4, 128] lhsT blocks of x^T
    xat = np.ascontiguousarray(
        x.transpose(0, 2, 1)                     # [B, 512, 4096]
         .reshape(B, 4, 128, NT, 128)            # [B, c, p, t, j]
         .transpose(0, 3, 2, 1, 4)               # [B, t, p, c, j]
    )
    xa = x

    if mode == "bf16":
        bf = ml_dtypes.bfloat16
        xa = xa.astype(bf)
        xat = xat.astype(bf)
    else:
        xa = np.ascontiguousarray(xa)
    return xa, xat, khatT, rhat, gext, augblk


def kernel(x, Wq1_w, Wq1_b, Wq2_w, Wq2_b, WR_w, WR_b):
    x = np.asarray(x, dtype=np.float32)
    args = [np.asarray(a, dtype=np.float32)
            for a in (Wq1_w, Wq1_b, Wq2_w, Wq2_b, WR_w, WR_b)]
    xa, xat, khatT, rhat, gext, augblk = _prep_host(x, *args, MODE)

    nc = _build(MODE)
    in_maps = [
        {"xa": xa[b], "xat": xat[b], "khatT": khatT, "rhat": rhat,
         "gext": gext[b], "augblk": augblk[b]}
        for b in range(B)
    ]
    res = run_bass_kernel_spmd(nc, in_maps, core_ids=list(range(N_CORES)))
    return np.stack([res.results[b]["out"] for b in range(B)])
